# revision 54
# baseline (speedup 1.0000x reference)
"""Bass/Tile multi-head-attention kernel for Trainium2, SPMD over 8 NeuronCores.

Sharding: core c = bs*2 + hh  (batch-parallel x head-tensor-parallel).
Each core computes 4 of the 8 heads for one batch over ALL 2048 queries,
producing a partial WO output; the host sums the pair (the post-WO
all-reduce) and transposes.  Host glue: slices/transposes/casts/pair-add.

Device math per core (bs, heads hh*4..hh*4+3):
  QpT = (WQh^T)-contract (qm . Q)^T   [256 d, 2048 q]   (mask in PSUM evac)
  KpT = ...                            [256 d, 2048 k]
  Vp  = (km . V) proj                  [k, 4, 65] ones col at 64
  per (h, kc, qhalf):
    S^T[k, q] = KpT_h^T-slice . QpT_h  (PE, contraction d=64)
    E = exp(S^T / 8)                   (ACT, PSUM->SBUF bf16)
    EP = E * mask^T                    (DVE bf16 2x)
  per (h, qc of 128 q):  psy[q, 65] += EP^T-slice . [Vp_h | 1]  (PE, 16 kc)
    yq[q, d] = psy[:, 0:64] * (km_q / psy[:, 64])   (DVE recip + Pool evac)
  yq head-pairs --XBAR DMA transpose--> Y^T [128 d, q]
  out^T = WOh^T-contract . Y^T         (PE) -> DRAM [e, q] f32 partial
"""

import numpy as np
import ml_dtypes

import concourse.bass as bass
import concourse.bacc as bacc
import concourse.mybir as mybir
import concourse.tile as tile
from concourse import bass_utils

BS, N, D, H = 4, 2048, 512, 8
HL = 4               # heads per core
DK = 64
DL = HL * DK         # 256 d-dims per core
NCORES = 8
KC = N // 128        # 16 key chunks
QB = 512             # projection / S matmul free block
F32 = mybir.dt.float32
BF16 = mybir.dt.bfloat16
NP_BF16 = ml_dtypes.bfloat16

FLAGS = {
    "warmup_mms": 12,
    "psy_banks": 2,    # concurrent psy accumulation banks
}


def _emit(nc, t):
    with tile.TileContext(nc) as tc:
        _emit_body(nc, tc, t)


def _emit_body(nc, tc, t):
    import contextlib
    ctx = contextlib.ExitStack()
    mult = mybir.AluOpType.mult
    with ctx:
        persist = ctx.enter_context(tc.tile_pool(name="persist", bufs=1))
        raw = ctx.enter_context(tc.tile_pool(name="raw", bufs=1))

        # ---- constants (q/k masks are pre-applied to Q/K/V on the host) ---
        kmc_sb = persist.tile([128, KC], F32, tag="kmc_sb")

        def load_consts():
            nc.sync.dma_start(out=kmc_sb[:], in_=t["kmc"].ap())

        # ---- weights (one DMA per weight tensor) --------------------------
        w_sb = {}
        for wname, cols, nt in (("wqt", DL, 4), ("wkt", DL, 4), ("wvt", DL, 4),
                                ("wot", D, 2)):
            w_sb[wname] = persist.tile([128, nt, cols], BF16, tag=wname, name=wname)

        def load_w(wname, eng=None):
            eng = eng or nc.sync
            eng.dma_start(out=w_sb[wname][:],
                          in_=t[wname].rearrange("(c p) d -> p c d", p=128))

        # ---- persistent activations --------------------------------------
        qpt_sb = [persist.tile([128, N], BF16, tag=f"qpt{i}", name=f"qpt{i}") for i in range(2)]
        kpt_sb = [persist.tile([128, N], BF16, tag=f"kpt{i}", name=f"kpt{i}") for i in range(2)]
        v_sb = [persist.tile([128, HL, DK + 1], BF16, tag=f"v{i}", name=f"v{i}")
                for i in range(KC)]
        yt_sb = [persist.tile([128, N], BF16, tag=f"yt{i}", name=f"yt{i}") for i in range(2)]

        qt_sb, kt_sb, vt_sb = [], [], []
        for nm, lst in (("qt", qt_sb), ("kt", kt_sb), ("vt", vt_sb)):
            for ct in range(4):
                lst.append(raw.tile([128, N], BF16, tag=f"{nm}{ct}", name=f"{nm}{ct}"))

        def load_raw(nm, lst, half, eng=None):
            eng = eng or nc.sync
            c0 = half * (N // 2)
            for ct in range(4):
                eng.dma_start(out=lst[ct][:, c0:c0 + N // 2],
                              in_=t[nm].ap()[ct * 128:(ct + 1) * 128, c0:c0 + N // 2])

        # ---- mask tiles: 4-kc groups, one DMA each ------------------------
        mpool = ctx.enter_context(tc.tile_pool(name="mts", bufs=1))
        mt_ap = t["mt"].ap()
        mts = {}

        def load_mtg(g, qh, eng=None):
            eng = eng or nc.sync
            mtile = mpool.tile([128, 4, 1024], BF16, tag="mt", name=f"mt{g}_{qh}",
                               bufs=4)
            off = (g * 512) * N + qh * 1024
            eng.dma_start(out=mtile[:],
                          in_=bass.AP(tensor=mt_ap.tensor, offset=mt_ap.offset + off,
                                      ap=[[N, 128], [128 * N, 4], [1, 1024]]))
            mts[(g, qh)] = mtile

        # Load order tracks first-use exec time through the serial DMA
        # pipe (~0.6us HWDGE issue + 0.36-2.9us transfer per DMA): the
        # column-halves feeding the first S tiles go first, mask groups are
        # interleaved at their consumption deadlines.
        # ACT's issue queue must drain fast (exp runs there): only the two
        # most-critical loads.  Late loads go through the Pool SWDGE.
        load_w("wqt", nc.scalar)
        load_raw("qt", qt_sb, 0, nc.scalar)
        load_consts()
        load_w("wkt")
        load_raw("kt", kt_sb, 0)
        load_mtg(0, 0)
        load_raw("kt", kt_sb, 1)
        load_mtg(1, 0)
        load_raw("vt", vt_sb, 0)
        load_w("wvt")
        load_mtg(2, 0)
        load_mtg(3, 0)
        load_raw("vt", vt_sb, 1)
        load_raw("qt", qt_sb, 1)
        load_w("wot")

        def emit_proj_q(dc, blk):
            ps = pool_y.tile([128, QB], F32, tag="psy", name="ps")
            for ct in range(4):
                nc.tensor.matmul(ps[:], w_sb["wqt"][:, ct, dc * 128:(dc + 1) * 128],
                                 qt_sb[ct][:, blk * QB:(blk + 1) * QB],
                                 start=(ct == 0), stop=(ct == 3))
            nc.vector.tensor_copy(out=qpt_sb[dc][:, blk * QB:(blk + 1) * QB],
                                   in_=ps[:])

        def emit_proj_k(dc, blk):
            ps = pool_y.tile([128, QB], F32, tag="psy", name="ps")
            for ct in range(4):
                nc.tensor.matmul(ps[:], w_sb["wkt"][:, ct, dc * 128:(dc + 1) * 128],
                                 kt_sb[ct][:, blk * QB:(blk + 1) * QB],
                                 start=(ct == 0), stop=(ct == 3))
            nc.vector.tensor_copy(out=kpt_sb[dc][:, blk * QB:(blk + 1) * QB],
                                   in_=ps[:])

        def emit_proj_v(kc):
            ps = pool_y.tile([128, QB], F32, tag="psy", name="psv")
            for ct in range(4):
                nc.tensor.matmul(ps[:, 0:DL], vt_sb[ct][:, kc * 128:(kc + 1) * 128],
                                 w_sb["wvt"][:, ct, :], start=(ct == 0), stop=(ct == 3))
            psap = ps[:]
            ps3 = bass.AP(tensor=psap.tensor, offset=psap.offset,
                          ap=[list(psap.ap[0]), [DK, HL], [1, DK]])
            nc.vector.tensor_copy(out=v_sb[kc][:, :, 0:DK], in_=ps3)
            nc.gpsimd.memset(v_sb[kc][:, :, DK:DK + 1], 1.0)

        # ---- attention pools ---------------------------------------------
        # PSUM: pss 3x2 banks + the shared psy ring (proj groups, AV psy
        # groups, and WO groups all rotate through the same 2 banks) = 8.
        pool_s = ctx.enter_context(tc.tile_pool(name="pss", bufs=3, space="PSUM"))
        pool_y = ctx.enter_context(tc.tile_pool(name="psy", bufs=int(FLAGS["psy_banks"]),
                                                space="PSUM"))
        etpool = ctx.enter_context(tc.tile_pool(name="ets", bufs=1))
        eppool = ctx.enter_context(tc.tile_pool(name="eps", bufs=1))
        yqpool = ctx.enter_context(tc.tile_pool(name="yqs", bufs=1))
        spool = ctx.enter_context(tc.tile_pool(name="smalls", bufs=4))
        opool = ctx.enter_context(tc.tile_pool(name="osb", bufs=3))

        # PE/ACT warm-up during the DMA ramp.
        nwarm = int(FLAGS["warmup_mms"])
        if nwarm:
            scratch = kpt_sb[0]
            nc.vector.memset(scratch[:, 0:QB], 0.0)
            ps_w = pool_y.tile([128, QB], F32, tag="psy", name="ps_w")
            for _ in range(nwarm):
                nc.tensor.matmul(ps_w[:], scratch[:, 0:128], scratch[:, 0:QB],
                                 start=True, stop=True)
            nc.scalar.activation(out=yt_sb[0][:, 0:QB], in_=ps_w[:],
                                 func=mybir.ActivationFunctionType.Exp, scale=0.125)

        eps = {}
        yqs = {}

        def av_unit(phi, qc):
            """One psy accumulation group: AV for (head-phase phi, 128-query
            chunk qc) + rowsum reciprocal + scaled evac into yq.  recip and
            evac both on DVE: one engine hop fewer in the bank-WAR chain."""
            qh, h = phi // 4, phi % 4
            hp, po = h // 2, (h % 2) * DK
            qa = qh * 8 + qc
            ps_y = pool_y.tile([128, QB], F32, tag="psy", name="psy")
            for j in range(KC):
                kc = (qc * 2 + j) % KC   # rotated kc order: staggered drain
                nc.tensor.matmul(ps_y[:, 0:DK + 1],
                                 eps[(h, kc, qh)][:, qc * 128:(qc + 1) * 128],
                                 v_sb[kc][:, h, 0:DK + 1],
                                 start=(j == 0), stop=(j == KC - 1))
            rec = spool.tile([128, 1], F32, tag="rec", name="rec")
            nc.vector.reciprocal(rec[:], ps_y[:, DK:DK + 1])
            if h % 2 == 0:
                yqs[(hp, qa)] = yqpool.tile([128, 128], BF16, tag=f"yq{qc}",
                                            name="yq", bufs=2)
            nc.vector.tensor_scalar(yqs[(hp, qa)][:, po:po + DK], ps_y[:, 0:DK],
                                    rec[:], kmc_sb[:, qa:qa + 1],
                                    op0=mult, op1=mult)

        def flush_tr(phi, qcs=range(8), split=False):
            """XBAR-transpose completed yq head-pair tiles of phase phi
            (odd head) into Y^T layout.  Issued only once their evacs are
            (nearly) done, so the issuing SEQ never parks long.  split=True
            alternates SP/ACT queues (tail: halves serial HWDGE issue)."""
            qh, h = phi // 4, phi % 4
            hp = h // 2
            for i, qc in enumerate(qcs):
                qa = qh * 8 + qc
                eng = nc.scalar if (split and i % 2) else nc.sync
                eng.dma_start_transpose(
                    out=yt_sb[hp][:, qa * 128:(qa + 1) * 128],
                    in_=yqs[(hp, qa)][:])

        ot_cur = {}

        def wo_unit(qb, ec, tail=False, split_store=False):
            ps = pool_y.tile([128, QB], F32, tag="psy", name="pso")
            for ct in range(2):
                nc.tensor.matmul(ps[:], w_sb["wot"][:, ct, ec * 128:(ec + 1) * 128],
                                 yt_sb[ct][:, qb * QB:(qb + 1) * QB],
                                 start=(ct == 0), stop=(ct == 1))
            if ec == 0:
                ot_cur[0] = opool.tile([128, 4, QB], F32, tag="ot", name="ot",
                                       bufs=1)
            ot = ot_cur[0]
            if tail:
                nc.scalar.copy(out=ot[:, ec, :], in_=ps[:])  # ACT idle in tail
            else:
                nc.vector.tensor_copy(out=ot[:, ec, :], in_=ps[:])
            oap = t["out_t"].ap()
            if split_store:
                # last block: per-slice stores overlap the evac cadence,
                # shortening the end-of-kernel DMA drain
                nc.gpsimd.dma_start(
                    out=oap[ec * 128:(ec + 1) * 128, qb * QB:(qb + 1) * QB],
                    in_=ot[:, ec, :])
            elif ec == 3:
                # one merged store per 512-query block: [p, c, q] -> rows c*128+p
                nc.gpsimd.dma_start(
                    out=bass.AP(tensor=oap.tensor, offset=oap.offset + qb * QB,
                                ap=[[N, 128], [128 * N, 4], [1, QB]]),
                    in_=ot[:])

        # ---- static background schedule: tile index -> emission closures --
        # AV(phi) units must finish by tile phi*16+(ep_bufs-1): later and the
        # ep pool slots they read get re-written by instructions emitted
        # after the reader, which WAR tracking cannot protect.  Proj
        # deadlines (first-use tile): K(0,b) by 4b; V* by AV(h0,qh0) at 23
        # (vt transfers land ~17us, so V units start at tile 7); K(1,0)/
        # Q(1,0..1) by h2 at 32 (scheduled early at 3..5 - they only need
        # kt/qt); K(1,1..3) by 36/40/44; Q(0,2..3) by 64; Q(1,2..3) by 96.
        from collections import defaultdict
        sched = defaultdict(list)
        sched[0].append(lambda: emit_proj_k(0, 1))
        sched[1].append(lambda: emit_proj_k(1, 0))
        sched[2].append(lambda: emit_proj_q(1, 0))
        sched[3].append(lambda: emit_proj_q(1, 1))
        sched[4].append(lambda: emit_proj_k(0, 2))
        sched[6].append(lambda: emit_proj_k(0, 3))
        for kc in range(KC):
            sched[7 + kc].append(lambda k=kc: emit_proj_v(k))
        for i, (kind, dc, blk) in enumerate(
                [("k", 1, 1), ("k", 1, 2), ("k", 1, 3),
                 ("q", 0, 2), ("q", 0, 3), ("q", 1, 2), ("q", 1, 3)]):
            f = emit_proj_k if kind == "k" else emit_proj_q
            sched[31 + i].append(lambda fn=f, d=dc, b=blk: fn(d, b))
        # phi=6 units sit at 120..127 (the exact ep-reuse ceiling): at +18
        # they wedge ahead of the final S tiles in PE order and stall the
        # exp stream ~6us through the psy/DVE ladder.
        for phi in range(7):
            base = 23 if phi == 0 else (120 if phi == 6 else phi * 16 + 18)
            for qc in range(8):
                sched[base + qc].append(lambda p=phi, q=qc: av_unit(p, q))
        for phi in (1, 3, 5):
            sched[phi * 16 + 26].append(lambda p=phi: flush_tr(p))
        for u in range(8):
            sched[90 + u].append(lambda b=u // 4, e=u % 4: wo_unit(b, e))

        # serial head-start: projections for the (h0, qh0) S sweep
        emit_proj_q(0, 0)
        emit_proj_q(0, 1)
        emit_proj_k(0, 0)

        tile_i = 0
        for qh in range(2):
            for h in range(HL):
                hp, po = h // 2, (h % 2) * DK
                for kc in range(KC):
                    ps_s = pool_s.tile([128, 1024], F32, tag="pss", name="ps_s")
                    for qq in range(2):
                        nc.tensor.matmul(ps_s[:, qq * QB:(qq + 1) * QB],
                                         kpt_sb[hp][po:po + DK, kc * 128:(kc + 1) * 128],
                                         qpt_sb[hp][po:po + DK,
                                                    qh * 1024 + qq * QB:
                                                    qh * 1024 + (qq + 1) * QB],
                                         start=True, stop=True)
                    for fn in sched.pop(tile_i, ()):
                        fn()
                    et = etpool.tile([128, 1024], BF16, tag="et", name="et", bufs=4)
                    nc.scalar.activation(out=et[:], in_=ps_s[:],
                                         func=mybir.ActivationFunctionType.Exp,
                                         scale=0.125)
                    ep = eppool.tile([128, 1024], BF16, tag="ep", name="ep", bufs=31)
                    nc.vector.tensor_mul(ep[:], et[:],
                                         mts[(kc // 4, qh)][:, kc % 4, :])
                    eps[(h, kc, qh)] = ep
                    # stage qh1 mask groups once qh0's are fully consumed
                    if qh == 0 and h == HL - 1 and kc % 4 == 3:
                        load_mtg(kc // 4, 1)
                    tile_i += 1
        # tail: last AV phase interleaved with its transposes and the qh1 WO
        for qc in range(4):
            av_unit(7, qc)
        flush_tr(7, range(4), split=True)
        for qc in range(4, 8):
            av_unit(7, qc)
            wo_unit(2, qc - 4, tail=True)
        flush_tr(7, range(4, 8), split=True)
        for ec in range(4):
            wo_unit(3, ec, tail=True, split_store=True)


_NC_CACHE = {}


def build():
    if "nc" in _NC_CACHE:
        return _NC_CACHE["nc"], _NC_CACHE["t"]
    nc = bacc.Bacc(None, target_bir_lowering=False, debug=False)
    t = {
        "qt": nc.dram_tensor("qt", [D, N], BF16, kind="ExternalInput"),
        "kt": nc.dram_tensor("kt", [D, N], BF16, kind="ExternalInput"),
        "vt": nc.dram_tensor("vt", [D, N], BF16, kind="ExternalInput"),
        "mt": nc.dram_tensor("mt", [N, N], BF16, kind="ExternalInput"),
        "kmc": nc.dram_tensor("kmc", [128, KC], F32, kind="ExternalInput"),
        "wqt": nc.dram_tensor("wqt", [D, DL], BF16, kind="ExternalInput"),
        "wkt": nc.dram_tensor("wkt", [D, DL], BF16, kind="ExternalInput"),
        "wvt": nc.dram_tensor("wvt", [D, DL], BF16, kind="ExternalInput"),
        "wot": nc.dram_tensor("wot", [DL, D], BF16, kind="ExternalInput"),
        "out_t": nc.dram_tensor("out_t", [D, N], F32, kind="ExternalOutput"),
    }
    _emit(nc, t)
    nc.compile()
    _NC_CACHE["nc"] = nc
    _NC_CACHE["t"] = t
    return nc, t


def make_in_maps(Q, K, V, q_mas, k_mas, att_mas, WQ, WK, WV, WO):
    Q, K, V = (np.asarray(x, np.float32) for x in (Q, K, V))
    q_mas = np.asarray(q_mas, np.float32).reshape(BS, N)
    k_mas = np.asarray(k_mas, np.float32).reshape(BS, N)
    att_mas = np.asarray(att_mas, np.float32)
    wqt = np.ascontiguousarray(np.asarray(WQ, np.float32).T)
    wkt = np.ascontiguousarray(np.asarray(WK, np.float32).T)
    wvt = np.ascontiguousarray(np.asarray(WV, np.float32).T)
    wot = np.ascontiguousarray(np.asarray(WO, np.float32).T)
    per_bs = {}
    for bs in range(BS):
        # q/k masks are 0/1 per token: folding them into Q/K/V before the
        # projections is exact (masking commutes with x @ W.T)
        qm = q_mas[bs][:, None]
        km = k_mas[bs][:, None]
        per_bs[bs] = {
            "qt": np.ascontiguousarray((Q[bs] * qm).T).astype(NP_BF16),
            "kt": np.ascontiguousarray((K[bs] * km).T).astype(NP_BF16),
            "vt": np.ascontiguousarray((V[bs] * km).T).astype(NP_BF16),
            "mt": np.ascontiguousarray(att_mas[bs].T).astype(NP_BF16),
            "kmc": np.ascontiguousarray(k_mas[bs].reshape(KC, 128).T).astype(np.float32),
        }
    in_maps = []
    for c in range(NCORES):
        bs, hh = c // 2, c % 2
        m = dict(per_bs[bs])
        m["wqt"] = np.ascontiguousarray(wqt[:, hh * DL:(hh + 1) * DL]).astype(NP_BF16)
        m["wkt"] = np.ascontiguousarray(wkt[:, hh * DL:(hh + 1) * DL]).astype(NP_BF16)
        m["wvt"] = np.ascontiguousarray(wvt[:, hh * DL:(hh + 1) * DL]).astype(NP_BF16)
        m["wot"] = np.ascontiguousarray(wot[hh * DL:(hh + 1) * DL, :]).astype(NP_BF16)
        in_maps.append(m)
    return in_maps


def kernel(Q, K, V, q_mas, k_mas, att_mas, WQ, WK, WV, WO):
    nc, _ = build()
    in_maps = make_in_maps(Q, K, V, q_mas, k_mas, att_mas, WQ, WK, WV, WO)
    res = bass_utils.run_bass_kernel_spmd(nc, in_maps, core_ids=list(range(NCORES)))
    out = np.empty((BS, N, D), np.float32)
    for bs in range(BS):
        acc = res.results[2 * bs]["out_t"] + res.results[2 * bs + 1]["out_t"]
        out[bs] = acc.T
    return out
